# revision 66
# baseline (speedup 1.0000x reference)
"""MHA (B=2, S=4096, D=512, H=8) on 8 trn2 cores — no collectives.

Core c: batch c//4, query rows = interleaved 128-blocks (block i of chunk
qc is global block 16qc+4i+c%4) so every core has an identical causal
structure. Each core projects full K^T/V (replicated within a batch),
Q^T for its rows, runs flash-style attention in transposed layout with
lag-1 PE pipelining, 2-PSUM-bank EXP batches, a host-shifted 16-entry
constant mask bank for diagonal tiles, ones-column softmax denominators,
reciprocal_approx_fast normalization, then its own output projection.
Host does all slicing/assembly; zero cross-core communication.
"""

import numpy as np

B, S, D, H, PD = 2, 4096, 512, 8, 64
P = 128
NCORES = 8
CPB = 4
DC = D // P        # 4
HP = 4             # head-pair blocks of 128 dims
QC = 512
NQC_C = 2          # q-chunks per core (1024 rows)
QR = NQC_C * QC
NKT = S // P       # 32
SC = 512
NSC = S // SC

_prog_cache = {}


def _build(mode: str):
    import concourse.mybir as mybir
    import concourse.tile as tile
    from concourse import bacc

    f32 = mybir.dt.float32
    f32r = mybir.dt.float32r
    bf16 = mybir.dt.bfloat16
    Exp = mybir.ActivationFunctionType.Exp

    nc = bacc.Bacc(debug=False, target_bir_lowering=False)

    xbT_d = nc.declare_dram_parameter("xbT", [P, DC, S], f32r, isOutput=False)
    xqT_d = nc.declare_dram_parameter("xqT", [P, DC, QR], f32r, isOutput=False)
    wq_d = nc.declare_dram_parameter("wq", [P, DC, D], f32r, isOutput=False)
    wk_d = nc.declare_dram_parameter("wk", [P, DC, D], f32r, isOutput=False)
    wv_d = nc.declare_dram_parameter("wv", [P, DC, D], f32r, isOutput=False)
    wo_d = nc.declare_dram_parameter("wo", [P, DC, D], f32r, isOutput=False)
    bq_d = nc.declare_dram_parameter("bq", [P, HP], f32, isOutput=False)
    bk_d = nc.declare_dram_parameter("bk", [P, HP], f32, isOutput=False)
    bv_d = nc.declare_dram_parameter("bv", [P, D], f32, isOutput=False)
    bo_d = nc.declare_dram_parameter("bo", [P, D], f32, isOutput=False)
    onesc_d = nc.declare_dram_parameter("onesc", [1, PD], f32r, isOutput=False)
    if mode == "tril":
        bank_d = nc.declare_dram_parameter("bank", [P, 16, QC], bf16,
                                           isOutput=False)
    out_d = nc.declare_dram_parameter("out", [NQC_C, 4, P, D], f32,
                                      isOutput=True)

    with tile.TileContext(nc) as tc, nc.allow_low_precision(
            reason="float32r tiles are 4-byte fp32; PE rounds reads only"):
        with (
            tc.tile_pool(name="const", bufs=1) as constp,
            tc.tile_pool(name="preB", bufs=1) as preB,
            tc.tile_pool(name="qk", bufs=2, space="PSUM") as qkps,
            tc.tile_pool(name="pj", bufs=2, space="PSUM") as pjps,
            tc.tile_pool(name="pv", bufs=2, space="PSUM") as pvps,
        ):
            kt = constp.tile([P, HP, S], bf16, tag="kt")
            vts = constp.tile([P, NKT, H, PD + 1], bf16, tag="vts")
            nc.vector.memset(vts[:, :, :, PD:PD + 1], 1.0)
            qt = constp.tile([P, HP, NQC_C, QC], bf16, tag="qt")
            wo = constp.tile([P, DC, D], f32r, tag="wo")
            bv = constp.tile([P, D], f32, tag="bv")
            bo = constp.tile([P, D], f32, tag="bo")
            bq = constp.tile([P, HP], f32, tag="bq")
            bk = constp.tile([P, HP], f32, tag="bk")
            onesc = constp.tile([1, PD], f32r, tag="onesc")

            wk = preB.tile([P, DC, D], f32r, tag="wk")
            wv = preB.tile([P, DC, D], f32r, tag="wv")
            xbtB = preB.tile([P, DC, S // 2], f32r, tag="xbtB")

            nc.sync.dma_start(bk[:], bk_d[:])
            # per-dc pieces: the first kproj matmul needs only dc=0 slices
            for dc in range(DC):
                nc.sync.dma_start(wk[:, dc], wk_d[:, dc])
            # dummy EXP: loads the ACT table during the (ACT-idle) prefix
            # instead of stalling the first attention softmax
            junk = constp.tile([1, HP], f32, tag="junk")
            nc.scalar.activation(junk[:], bk[0:1, :], Exp, scale=0.125)

            nkt_of = [16, 32] if mode == "tril" else [NKT, NKT]

            def kproj(sc, hp, xsrc, c0):
                psk = pjps.tile([P, QC], f32, tag="pj")
                for dc in range(DC):
                    nc.tensor.matmul(
                        psk[:], wk[:, dc, hp * P:(hp + 1) * P],
                        xsrc[:, dc, sc * SC - c0:(sc + 1) * SC - c0],
                        start=(dc == 0), stop=(dc == DC - 1))
                nc.vector.tensor_scalar_add(
                    kt[:, hp, sc * SC:(sc + 1) * SC], psk[:],
                    bk[:, hp:hp + 1])

            def vproj(st, xsrc, c0):
                psv = pjps.tile([P, QC], f32, tag="pj")
                for dc in range(DC):
                    nc.tensor.matmul(
                        psv[:], xsrc[:, dc, st * P - c0:(st + 1) * P - c0],
                        wv[:, dc, :],
                        start=(dc == 0), stop=(dc == DC - 1))
                nc.vector.tensor_add(
                    out=vts[:, st, :, 0:PD],
                    in0=psv[:].rearrange("p (h d) -> p h d", h=H),
                    in1=bv[:].rearrange("p (h d) -> p h d", h=H))

            # ---- prefix: K/V for seq chunks 0-3, Q for all own rows ----
            with tc.tile_pool(name="preA", bufs=1) as preA:
                wq = preA.tile([P, DC, D], f32r, tag="wq")
                xbtA = preA.tile([P, DC, S // 2], f32r, tag="xbtA")
                xqt = preA.tile([P, DC, QR], f32r, tag="xqt")
                for dc in range(DC):
                    nc.scalar.dma_start(xbtA[:, dc, 0:SC],
                                        xbT_d[:, dc, 0:SC])
                for t, d_ in [(wv, wv_d), (bv, bv_d)]:
                    nc.sync.dma_start(t[:], d_[:])
                for sc in range(1, NSC // 2):
                    nc.sync.dma_start(
                        xbtA[:, :, sc * SC:(sc + 1) * SC],
                        xbT_d[:, :, sc * SC:(sc + 1) * SC])
                nc.sync.dma_start(xqt[:], xqT_d[:])
                for t, d_ in [(wq, wq_d), (bq, bq_d), (wo, wo_d),
                              (bo, bo_d), (onesc, onesc_d)]:
                    nc.sync.dma_start(t[:], d_[:])
                for sc in range(NSC // 2, NSC):
                    nc.sync.dma_start(
                        xbtB[:, :, (sc - NSC // 2) * SC:
                             (sc - NSC // 2 + 1) * SC],
                        xbT_d[:, :, sc * SC:(sc + 1) * SC])

                for sc in range(NSC // 2):
                    for hp in range(HP):
                        kproj(sc, hp, xbtA, 0)
                    for st in range(4 * sc, 4 * sc + 4):
                        vproj(st, xbtA, 0)
                for qc in range(NQC_C):
                    for hp in range(HP):
                        psq = pjps.tile([P, QC], f32, tag="pj")
                        for dc in range(DC):
                            nc.tensor.matmul(
                                psq[:], wq[:, dc, hp * P:(hp + 1) * P],
                                xqt[:, dc, qc * QC:(qc + 1) * QC],
                                start=(dc == 0), stop=(dc == DC - 1))
                        nc.vector.tensor_scalar_add(
                            qt[:, hp, qc, :], psq[:], bq[:, hp:hp + 1])

            def kv_units(scs):
                us = []
                for sc in scs:
                    for hp in range(HP):
                        us.append(
                            (lambda a, b: lambda: kproj(a, b, xbtB,
                                                        S // 2))(sc, hp))
                    for st in range(4 * sc, 4 * sc + 4):
                        us.append(
                            (lambda a: lambda: vproj(a, xbtB,
                                                     S // 2))(st))
                return us

            if mode == "tril":
                pending = kv_units([4, 5])
            else:
                pending = kv_units([4, 5, 6, 7])
            # cover the pool-transition drain with barrier-free PE work
            for u in pending[:6]:
                u()
            pending = pending[6:]
            if mode != "tril":
                for u in pending:
                    u()
                pending = []

            # ---- attention pools live in the space preA released ----
            with (
                tc.tile_pool(name="pt", bufs=8) as ptp,
                tc.tile_pool(name="at", bufs=2) as atp,
                tc.tile_pool(name="osb", bufs=2) as osbp,
                tc.tile_pool(name="bcs", bufs=2) as bcsp,
                tc.tile_pool(name="rcp", bufs=2) as rcpp,
                tc.tile_pool(name="bankp", bufs=1) as bankp,
            ):
                if mode == "tril":
                    bank = bankp.tile([P, 16, QC], bf16, tag="bank")
                    nc.scalar.dma_start(bank[:], bank_d[:])

                def outproj_units(qc, at_tile):
                    def rt_unit(rt):
                        def f():
                            psf = pjps.tile([P, QC], f32, tag="pj")
                            for dc in range(DC):
                                nc.tensor.matmul(
                                    psf[:],
                                    at_tile[:, dc, rt * P:(rt + 1) * P],
                                    wo[:, dc, :],
                                    start=(dc == 0), stop=(dc == DC - 1))
                            osb = osbp.tile([P, D], f32, tag="osb")
                            nc.vector.tensor_add(out=osb[:], in0=psf[:],
                                                 in1=bo[:])
                            nc.sync.dma_start(out_d[qc, rt], osb[:])
                        return f
                    return [rt_unit(rt) for rt in range(4)]


                for qc in range(NQC_C):
                    if qc == 1 and mode == "tril":
                        pending = kv_units([6, 7]) + pending
                    nkt_c = nkt_of[qc]
                    npairs_h = nkt_c // 2
                    npairs = H * npairs_h
                    at_tile = atp.tile([P, DC, QC], f32r, tag="at")
                    pair_idx = 0
                    fill_emitted = 0
                    for h in range(H):
                        po = (h % 2) * PD
                        hp = h // 2
                        pvt = pvps.tile([PD + 1, QC], f32, tag="pv",
                                        name=f"pv{qc}_{h}")
                        prev_pt2 = None
                        for i in range(npairs_h):
                            qkt = qkps.tile([P, 2, QC], f32, tag="qk")
                            for j in range(2):
                                kc = 2 * i + j
                                nc.tensor.matmul(
                                    qkt[:, j, :],
                                    kt[po:po + PD, hp, kc * P:(kc + 1) * P],
                                    qt[po:po + PD, hp, qc, :],
                                    start=True, stop=True)
                            banded = (mode == "tril"
                                      and 2 * i >= 16 * qc)
                            pt2 = ptp.tile([P, 2, QC], bf16, tag="pt")
                            if banded:
                                pr2 = ptp.tile([P, 2, QC], bf16, tag="pt")
                                nc.scalar.activation(pr2[:], qkt[:], Exp,
                                                     scale=0.125)
                                j0 = 2 * i - 16 * qc
                                nc.vector.tensor_mul(
                                    out=pt2[:], in0=pr2[:],
                                    in1=bank[:, j0:j0 + 2, :])
                            else:
                                nc.scalar.activation(pt2[:], qkt[:], Exp,
                                                     scale=0.125)
                            if prev_pt2 is not None:
                                for j in range(2):
                                    kc = 2 * (i - 1) + j
                                    nc.tensor.matmul(
                                        pvt[:], vts[:, kc, h, :],
                                        prev_pt2[:, j, :],
                                        start=(kc == 0), stop=False,
                                        skip_group_check=True)
                            prev_pt2 = pt2
                            pair_idx += 1
                            if qc == 1 and mode == "tril":
                                want = min(len(pending), 2 + pair_idx)
                            else:
                                want = len(pending) * pair_idx // npairs
                            while fill_emitted < want:
                                pending[fill_emitted]()
                                fill_emitted += 1
                        for j in range(2):
                            kc = 2 * (npairs_h - 1) + j
                            nc.tensor.matmul(
                                pvt[:], vts[:, kc, h, :], prev_pt2[:, j, :],
                                start=(kc == 0), stop=(kc == nkt_c - 1),
                                skip_group_check=True)
                        dn = rcpp.tile([1, QC], f32r, tag="rcp")
                        nc.vector.tensor_copy(out=dn[:],
                                              in_=pvt[PD:PD + 1, :])
                        bcp = pjps.tile([P, QC], f32, tag="pj")
                        nc.tensor.matmul(bcp[0:PD, :], onesc[:], dn[:],
                                         start=True, stop=True)
                        bcs = bcsp.tile([PD, QC], f32, tag="bcs")
                        nc.vector.reciprocal_approx_fast(
                            out=bcs[:], in_=bcp[0:PD, :])
                        nc.vector.tensor_mul(
                            out=at_tile[po:po + PD, hp, :],
                            in0=pvt[0:PD, :], in1=bcs[:])
                    for u in pending[fill_emitted:]:
                        u()
                    pending = outproj_units(qc, at_tile)
                for u in pending:
                    u()
    nc.finalize()
    return nc


def _get_prog(mode: str):
    if mode not in _prog_cache:
        _prog_cache[mode] = _build(mode)
    return _prog_cache[mode]


def _rows(c):
    r = c % CPB
    g = np.array([16 * qc + 4 * i + r for qc in range(NQC_C)
                  for i in range(4)])
    return (g[:, None] * P + np.arange(P)[None, :]).ravel()


def make_in_maps(inputs, mask, Wq, bq, Wk, bk, Wv, bv, Wo, bo):
    import ml_dtypes
    inputs = np.asarray(inputs, dtype=np.float32)
    mask = np.asarray(mask, dtype=np.float32)
    if not np.any(mask):
        mode = "none"
    elif np.array_equal(mask, np.triu(np.ones((S, S), dtype=np.float32), 1)):
        mode = "tril"
    else:
        raise ValueError("unsupported mask pattern")

    def warr(W):
        return np.ascontiguousarray(
            np.asarray(W, np.float32).reshape(DC, P, D).transpose(1, 0, 2))

    def barr(b_):
        return np.ascontiguousarray(
            np.asarray(b_, np.float32).reshape(HP, P).T)

    shared = {
        "wq": warr(Wq), "wk": warr(Wk), "wv": warr(Wv), "wo": warr(Wo),
        "bq": barr(bq), "bk": barr(bk),
        "bv": np.ascontiguousarray(
            np.broadcast_to(np.asarray(bv, np.float32), (P, D))),
        "bo": np.ascontiguousarray(
            np.broadcast_to(np.asarray(bo, np.float32), (P, D))),
        "onesc": np.ones((1, PD), dtype=np.float32),
    }
    xbTs = [np.ascontiguousarray(
        inputs[b].T.reshape(DC, P, S).transpose(1, 0, 2)) for b in range(B)]

    in_maps = []
    for c in range(NCORES):
        b, r = c // CPB, c % CPB
        rows = _rows(c)
        m = dict(shared)
        m["xbT"] = xbTs[b]
        m["xqT"] = np.ascontiguousarray(
            inputs[b][rows].T.reshape(DC, P, QR).transpose(1, 0, 2))
        if mode == "tril":
            # bank[p, j, i*128+q'] for s = j - r - 4i:
            # s<0 -> 1, s==0 -> (q' >= p), s>0 -> 0
            pp = np.arange(P)[:, None, None]
            jj = np.arange(16)[None, :, None]
            qq = np.arange(QC)[None, None, :]
            s = jj - r - 4 * (qq // P)
            qp = qq % P
            bank = np.where(s < 0, 1.0,
                            np.where(s > 0, 0.0,
                                     (qp >= pp).astype(np.float32)))
            m["bank"] = bank.astype(ml_dtypes.bfloat16)
        in_maps.append(m)
    return mode, in_maps


def assemble(results, mode):
    out = np.empty((B, S, D), dtype=np.float32)
    for c in range(NCORES):
        b = c // CPB
        out[b, _rows(c)] = results[c]["out"].reshape(QR, D)
    return out


def kernel(inputs, mask, Wq, bq, Wk, bk, Wv, bv, Wo, bo):
    from concourse.bass_utils import run_bass_kernel_spmd

    mode, in_maps = make_in_maps(inputs, mask, Wq, bq, Wk, bk, Wv, bv, Wo, bo)
    nc = _get_prog(mode)
    res = run_bass_kernel_spmd(nc, in_maps, core_ids=list(range(NCORES)))
    return assemble(res.results, mode)


# revision 67
# speedup vs baseline: 1.0140x; 1.0140x over previous
"""MHA (B=2, S=4096, D=512, H=8) on 8 trn2 cores — no collectives.

Core c: batch c//4, query rows = interleaved 128-blocks (block i of chunk
qc is global block 16qc+4i+c%4) so every core has an identical causal
structure. Each core projects full K^T/V (replicated within a batch),
Q^T for its rows, runs flash-style attention in transposed layout with
lag-1 PE pipelining, 2-PSUM-bank EXP batches, a host-shifted 16-entry
constant mask bank for diagonal tiles, ones-column softmax denominators,
reciprocal_approx_fast normalization, then its own output projection.
Host does all slicing/assembly; zero cross-core communication.
"""

import numpy as np

B, S, D, H, PD = 2, 4096, 512, 8, 64
P = 128
NCORES = 8
CPB = 4
DC = D // P        # 4
HP = 4             # head-pair blocks of 128 dims
QC = 512
NQC_C = 2          # q-chunks per core (1024 rows)
QR = NQC_C * QC
NKT = S // P       # 32
SC = 512
NSC = S // SC

_prog_cache = {}


def _build(mode: str):
    import concourse.mybir as mybir
    import concourse.tile as tile
    from concourse import bacc

    f32 = mybir.dt.float32
    f32r = mybir.dt.float32r
    bf16 = mybir.dt.bfloat16
    Exp = mybir.ActivationFunctionType.Exp

    nc = bacc.Bacc(debug=False, target_bir_lowering=False)

    xbT_d = nc.declare_dram_parameter("xbT", [P, DC, S], f32r, isOutput=False)
    xqT_d = nc.declare_dram_parameter("xqT", [P, DC, QR], f32r, isOutput=False)
    wq_d = nc.declare_dram_parameter("wq", [P, DC, D], f32r, isOutput=False)
    wk_d = nc.declare_dram_parameter("wk", [P, DC, D], f32r, isOutput=False)
    wv_d = nc.declare_dram_parameter("wv", [P, DC, D], f32r, isOutput=False)
    wo_d = nc.declare_dram_parameter("wo", [P, DC, D], f32r, isOutput=False)
    bq_d = nc.declare_dram_parameter("bq", [P, HP], f32, isOutput=False)
    bk_d = nc.declare_dram_parameter("bk", [P, HP], f32, isOutput=False)
    bv_d = nc.declare_dram_parameter("bv", [P, D], f32, isOutput=False)
    bo_d = nc.declare_dram_parameter("bo", [P, D], f32, isOutput=False)
    onesc_d = nc.declare_dram_parameter("onesc", [1, PD], f32r, isOutput=False)
    if mode == "tril":
        bank_d = nc.declare_dram_parameter("bank", [P, 16, QC], bf16,
                                           isOutput=False)
    out_d = nc.declare_dram_parameter("out", [NQC_C, 4, P, D], f32,
                                      isOutput=True)

    with tile.TileContext(nc) as tc, nc.allow_low_precision(
            reason="float32r tiles are 4-byte fp32; PE rounds reads only"):
        with (
            tc.tile_pool(name="const", bufs=1) as constp,
            tc.tile_pool(name="preB", bufs=1) as preB,
            tc.tile_pool(name="qk", bufs=2, space="PSUM") as qkps,
            tc.tile_pool(name="pj", bufs=2, space="PSUM") as pjps,
            tc.tile_pool(name="pv", bufs=2, space="PSUM") as pvps,
        ):
            kt = constp.tile([P, HP, S], bf16, tag="kt")
            vts = constp.tile([P, NKT, H, PD + 1], bf16, tag="vts")
            nc.vector.memset(vts[:, :, :, PD:PD + 1], 1.0)
            qt = constp.tile([P, HP, NQC_C, QC], bf16, tag="qt")
            wo = constp.tile([P, DC, D], f32r, tag="wo")
            bv = constp.tile([P, D], f32, tag="bv")
            bo = constp.tile([P, D], f32, tag="bo")
            bq = constp.tile([P, HP], f32, tag="bq")
            bk = constp.tile([P, HP], f32, tag="bk")
            onesc = constp.tile([1, PD], f32r, tag="onesc")

            wk = preB.tile([P, DC, D], f32r, tag="wk")
            wv = preB.tile([P, DC, D], f32r, tag="wv")
            xbtB = preB.tile([P, DC, S // 2], f32r, tag="xbtB")

            for t, d_ in [(wk, wk_d), (bk, bk_d)]:
                nc.sync.dma_start(t[:], d_[:])
            # dummy EXP: loads the ACT table during the (ACT-idle) prefix
            # instead of stalling the first attention softmax
            junk = constp.tile([1, HP], f32, tag="junk")
            nc.scalar.activation(junk[:], bk[0:1, :], Exp, scale=0.125)

            nkt_of = [16, 32] if mode == "tril" else [NKT, NKT]

            def kproj(sc, hp, xsrc, c0):
                psk = pjps.tile([P, QC], f32, tag="pj")
                for dc in range(DC):
                    nc.tensor.matmul(
                        psk[:], wk[:, dc, hp * P:(hp + 1) * P],
                        xsrc[:, dc, sc * SC - c0:(sc + 1) * SC - c0],
                        start=(dc == 0), stop=(dc == DC - 1))
                nc.vector.tensor_scalar_add(
                    kt[:, hp, sc * SC:(sc + 1) * SC], psk[:],
                    bk[:, hp:hp + 1])

            def vproj(st, xsrc, c0):
                psv = pjps.tile([P, QC], f32, tag="pj")
                for dc in range(DC):
                    nc.tensor.matmul(
                        psv[:], xsrc[:, dc, st * P - c0:(st + 1) * P - c0],
                        wv[:, dc, :],
                        start=(dc == 0), stop=(dc == DC - 1))
                nc.vector.tensor_add(
                    out=vts[:, st, :, 0:PD],
                    in0=psv[:].rearrange("p (h d) -> p h d", h=H),
                    in1=bv[:].rearrange("p (h d) -> p h d", h=H))

            # ---- prefix: K/V for seq chunks 0-3, Q for all own rows ----
            with tc.tile_pool(name="preA", bufs=1) as preA:
                wq = preA.tile([P, DC, D], f32r, tag="wq")
                xbtA = preA.tile([P, DC, S // 2], f32r, tag="xbtA")
                xqt = preA.tile([P, DC, QR], f32r, tag="xqt")
                nc.scalar.dma_start(xbtA[:, :, 0:SC],
                                    xbT_d[:, :, 0:SC])
                for t, d_ in [(wv, wv_d), (bv, bv_d)]:
                    nc.sync.dma_start(t[:], d_[:])
                for sc in range(1, NSC // 2):
                    nc.sync.dma_start(
                        xbtA[:, :, sc * SC:(sc + 1) * SC],
                        xbT_d[:, :, sc * SC:(sc + 1) * SC])
                nc.sync.dma_start(xqt[:], xqT_d[:])
                for t, d_ in [(wq, wq_d), (bq, bq_d), (wo, wo_d),
                              (bo, bo_d), (onesc, onesc_d)]:
                    nc.sync.dma_start(t[:], d_[:])
                for sc in range(NSC // 2, NSC):
                    nc.sync.dma_start(
                        xbtB[:, :, (sc - NSC // 2) * SC:
                             (sc - NSC // 2 + 1) * SC],
                        xbT_d[:, :, sc * SC:(sc + 1) * SC])

                for sc in range(NSC // 2):
                    for hp in range(HP):
                        kproj(sc, hp, xbtA, 0)
                    for st in range(4 * sc, 4 * sc + 4):
                        vproj(st, xbtA, 0)
                for qc in range(NQC_C):
                    for hp in range(HP):
                        psq = pjps.tile([P, QC], f32, tag="pj")
                        for dc in range(DC):
                            nc.tensor.matmul(
                                psq[:], wq[:, dc, hp * P:(hp + 1) * P],
                                xqt[:, dc, qc * QC:(qc + 1) * QC],
                                start=(dc == 0), stop=(dc == DC - 1))
                        nc.vector.tensor_scalar_add(
                            qt[:, hp, qc, :], psq[:], bq[:, hp:hp + 1])

            def kv_units(scs):
                us = []
                for sc in scs:
                    for hp in range(HP):
                        us.append(
                            (lambda a, b: lambda: kproj(a, b, xbtB,
                                                        S // 2))(sc, hp))
                    for st in range(4 * sc, 4 * sc + 4):
                        us.append(
                            (lambda a: lambda: vproj(a, xbtB,
                                                     S // 2))(st))
                return us

            if mode == "tril":
                pending = kv_units([4, 5])
            else:
                pending = kv_units([4, 5, 6, 7])
            # cover the pool-transition drain with barrier-free PE work
            for u in pending[:6]:
                u()
            pending = pending[6:]
            if mode != "tril":
                for u in pending:
                    u()
                pending = []

            # ---- attention pools live in the space preA released ----
            with (
                tc.tile_pool(name="pt", bufs=8) as ptp,
                tc.tile_pool(name="at", bufs=2) as atp,
                tc.tile_pool(name="osb", bufs=2) as osbp,
                tc.tile_pool(name="bcs", bufs=2) as bcsp,
                tc.tile_pool(name="rcp", bufs=2) as rcpp,
                tc.tile_pool(name="bankp", bufs=1) as bankp,
            ):
                if mode == "tril":
                    bank = bankp.tile([P, 16, QC], bf16, tag="bank")
                    nc.scalar.dma_start(bank[:], bank_d[:])

                def outproj_units(qc, at_tile):
                    def rt_unit(rt):
                        def f():
                            psf = pjps.tile([P, QC], f32, tag="pj")
                            for dc in range(DC):
                                nc.tensor.matmul(
                                    psf[:],
                                    at_tile[:, dc, rt * P:(rt + 1) * P],
                                    wo[:, dc, :],
                                    start=(dc == 0), stop=(dc == DC - 1))
                            osb = osbp.tile([P, D], f32, tag="osb")
                            nc.vector.tensor_add(out=osb[:], in0=psf[:],
                                                 in1=bo[:])
                            nc.sync.dma_start(out_d[qc, rt], osb[:])
                        return f
                    return [rt_unit(rt) for rt in range(4)]


                for qc in range(NQC_C):
                    if qc == 1 and mode == "tril":
                        pending = kv_units([6, 7]) + pending
                    nkt_c = nkt_of[qc]
                    npairs_h = nkt_c // 2
                    npairs = H * npairs_h
                    at_tile = atp.tile([P, DC, QC], f32r, tag="at")
                    pair_idx = 0
                    fill_emitted = 0
                    for h in range(H):
                        po = (h % 2) * PD
                        hp = h // 2
                        pvt = pvps.tile([PD + 1, QC], f32, tag="pv",
                                        name=f"pv{qc}_{h}")
                        prev_pt2 = None
                        for i in range(npairs_h):
                            qkt = qkps.tile([P, 2, QC], f32, tag="qk")
                            for j in range(2):
                                kc = 2 * i + j
                                nc.tensor.matmul(
                                    qkt[:, j, :],
                                    kt[po:po + PD, hp, kc * P:(kc + 1) * P],
                                    qt[po:po + PD, hp, qc, :],
                                    start=True, stop=True)
                            banded = (mode == "tril"
                                      and 2 * i >= 16 * qc)
                            pt2 = ptp.tile([P, 2, QC], bf16, tag="pt")
                            if banded:
                                pr2 = ptp.tile([P, 2, QC], bf16, tag="pt")
                                nc.scalar.activation(pr2[:], qkt[:], Exp,
                                                     scale=0.125)
                                j0 = 2 * i - 16 * qc
                                nc.vector.tensor_mul(
                                    out=pt2[:], in0=pr2[:],
                                    in1=bank[:, j0:j0 + 2, :])
                            else:
                                nc.scalar.activation(pt2[:], qkt[:], Exp,
                                                     scale=0.125)
                            if prev_pt2 is not None:
                                for j in range(2):
                                    kc = 2 * (i - 1) + j
                                    nc.tensor.matmul(
                                        pvt[:], vts[:, kc, h, :],
                                        prev_pt2[:, j, :],
                                        start=(kc == 0), stop=False,
                                        skip_group_check=True)
                            prev_pt2 = pt2
                            pair_idx += 1
                            if qc == 1 and mode == "tril":
                                want = min(len(pending), 2 + pair_idx)
                            else:
                                want = len(pending) * pair_idx // npairs
                            while fill_emitted < want:
                                pending[fill_emitted]()
                                fill_emitted += 1
                        for j in range(2):
                            kc = 2 * (npairs_h - 1) + j
                            nc.tensor.matmul(
                                pvt[:], vts[:, kc, h, :], prev_pt2[:, j, :],
                                start=(kc == 0), stop=(kc == nkt_c - 1),
                                skip_group_check=True)
                        dn = rcpp.tile([1, QC], f32r, tag="rcp")
                        nc.vector.tensor_copy(out=dn[:],
                                              in_=pvt[PD:PD + 1, :])
                        bcp = pjps.tile([P, QC], f32, tag="pj")
                        nc.tensor.matmul(bcp[0:PD, :], onesc[:], dn[:],
                                         start=True, stop=True)
                        bcs = bcsp.tile([PD, QC], f32, tag="bcs")
                        nc.vector.reciprocal_approx_fast(
                            out=bcs[:], in_=bcp[0:PD, :])
                        nc.vector.tensor_mul(
                            out=at_tile[po:po + PD, hp, :],
                            in0=pvt[0:PD, :], in1=bcs[:])
                    for u in pending[fill_emitted:]:
                        u()
                    pending = outproj_units(qc, at_tile)
                for u in pending:
                    u()
    nc.finalize()
    return nc


def _get_prog(mode: str):
    if mode not in _prog_cache:
        _prog_cache[mode] = _build(mode)
    return _prog_cache[mode]


def _rows(c):
    r = c % CPB
    g = np.array([16 * qc + 4 * i + r for qc in range(NQC_C)
                  for i in range(4)])
    return (g[:, None] * P + np.arange(P)[None, :]).ravel()


def make_in_maps(inputs, mask, Wq, bq, Wk, bk, Wv, bv, Wo, bo):
    import ml_dtypes
    inputs = np.asarray(inputs, dtype=np.float32)
    mask = np.asarray(mask, dtype=np.float32)
    if not np.any(mask):
        mode = "none"
    elif np.array_equal(mask, np.triu(np.ones((S, S), dtype=np.float32), 1)):
        mode = "tril"
    else:
        raise ValueError("unsupported mask pattern")

    def warr(W):
        return np.ascontiguousarray(
            np.asarray(W, np.float32).reshape(DC, P, D).transpose(1, 0, 2))

    def barr(b_):
        return np.ascontiguousarray(
            np.asarray(b_, np.float32).reshape(HP, P).T)

    shared = {
        "wq": warr(Wq), "wk": warr(Wk), "wv": warr(Wv), "wo": warr(Wo),
        "bq": barr(bq), "bk": barr(bk),
        "bv": np.ascontiguousarray(
            np.broadcast_to(np.asarray(bv, np.float32), (P, D))),
        "bo": np.ascontiguousarray(
            np.broadcast_to(np.asarray(bo, np.float32), (P, D))),
        "onesc": np.ones((1, PD), dtype=np.float32),
    }
    xbTs = [np.ascontiguousarray(
        inputs[b].T.reshape(DC, P, S).transpose(1, 0, 2)) for b in range(B)]

    in_maps = []
    for c in range(NCORES):
        b, r = c // CPB, c % CPB
        rows = _rows(c)
        m = dict(shared)
        m["xbT"] = xbTs[b]
        m["xqT"] = np.ascontiguousarray(
            inputs[b][rows].T.reshape(DC, P, QR).transpose(1, 0, 2))
        if mode == "tril":
            # bank[p, j, i*128+q'] for s = j - r - 4i:
            # s<0 -> 1, s==0 -> (q' >= p), s>0 -> 0
            pp = np.arange(P)[:, None, None]
            jj = np.arange(16)[None, :, None]
            qq = np.arange(QC)[None, None, :]
            s = jj - r - 4 * (qq // P)
            qp = qq % P
            bank = np.where(s < 0, 1.0,
                            np.where(s > 0, 0.0,
                                     (qp >= pp).astype(np.float32)))
            m["bank"] = bank.astype(ml_dtypes.bfloat16)
        in_maps.append(m)
    return mode, in_maps


def assemble(results, mode):
    out = np.empty((B, S, D), dtype=np.float32)
    for c in range(NCORES):
        b = c // CPB
        out[b, _rows(c)] = results[c]["out"].reshape(QR, D)
    return out


def kernel(inputs, mask, Wq, bq, Wk, bk, Wv, bv, Wo, bo):
    from concourse.bass_utils import run_bass_kernel_spmd

    mode, in_maps = make_in_maps(inputs, mask, Wq, bq, Wk, bk, Wv, bv, Wo, bo)
    nc = _get_prog(mode)
    res = run_bass_kernel_spmd(nc, in_maps, core_ids=list(range(NCORES)))
    return assemble(res.results, mode)
